# revision 47
# baseline (speedup 1.0000x reference)
"""Trainium2 Bass kernel for the ChebConv GNN problem
(nn_ChebConvConvolutional): 2x GCNConv + 1x ChebConv(K=3), N=10000 nodes,
E=160000 edges, F=512, celu activations.

Strategy (8 NeuronCores, SPMD):
  * Nodes are sharded 1250/core (padded to 1280). Edges are sharded by
    destination core and grouped into 128-dest tiles; per dest-tile the
    source nodes are deduplicated and the edge weights are baked into dense
    [128 src x 128 dst] one-hot "S" matrices (GCN self-loops folded in as
    edges with value dinv^2, Cheb normalization negated so the scatter
    directly produces lhat).
  * Every graph op is computed aggregate-first: h = celu((A @ x) @ W + b);
    layers end with a small AllGather of the core's 1280-row bf16 slice.
  * Layer 1's gather indices are host-known, so its message rows are
    pre-gathered on the host and streamed with static HWDGE DMA (no
    SWDGE descriptor generation). Later passes dma_gather from the
    AllGathered tables; the tensor engine computes
    ps[d, f] += S[e, d].T @ msgs[e, f], then dense GEMMs, and
    celu = max(z,0) + min(exp(z)-1, 0) runs on ACT + DVE.
  * SBUF is time-shared via scoped tile pools: the GCN S tiles release
    after layer 2 and the cheb S tiles load into the freed zone during
    the pq AllGather; message pools are triple/quadruple buffered.
  * ChebConv K=3 uses the commuted form (lhat is row-mixing, W col-mixing):
        out = celu(h2 @ Wa + lhat(h2 @ Wb) + lhat(lhat(h2 @ Wc2)) + bc)
    with Wa = Wk0 - Wk2, Wb = Wk1, Wc2 = 2*Wk2. After layer 2 each core
    projects its h2 tiles to pq = h2 @ [Wb | Wc2] (512 wide), AllGathers pq
    once; pass A scatters pq into [lhat_p | lhat_q], keeps lhat_p on chip
    and AllGathers r = lhat_q (256 wide); pass B scatters r (half-width
    gather) and finishes out = celu(psum + h2 @ Wa + lhat_p + bc) in psum.
"""
import numpy as np
import ml_dtypes

import concourse.bacc as bacc
import concourse.mybir as mybir
import concourse.tile as tile
from concourse import library_config
from concourse.bass_utils import run_bass_kernel_spmd
from concourse.tile import add_dep_helper

BF16 = ml_dtypes.bfloat16
FP32 = mybir.dt.float32
BF16D = mybir.dt.bfloat16
FP8D = mybir.dt.float8e4
I16 = mybir.dt.int16

P = 8            # cores
N = 10000        # nodes
NPC = N // P     # nodes per core
NPAD = 1280      # padded nodes per core
NTOT = NPAD * P
F = 512          # feature width of x / h1 / h2
DOUT = 256
DT = 128         # dests per dest tile
NDT = NPAD // DT # dest tiles per core
KC = F // 128    # contraction chunks (4)


# ----------------------------------------------------------------- host prep

def _to_padded_id(n):
    """Global node id -> row in the AllGather global layout: [P ranks][NPAD]."""
    r = n // NPC
    return r * NPAD + (n % NPC)


def _build_edge_tiles(src, dst, val):
    """Shard by dest core, tile by 128 dests, dedup sources per tile.
    Own-rank sources are ordered first so their slabs can be aggregated
    from the Local staging tensor during the AllGather window.

    Returns (ETo, ETr: per-tile own/remote slab counts,
             idx_own [P, To, 128] int32 LOCAL row ids,
             idx_rem [P, Tr, 128] int32 padded global ids,
             S [P, To+Tr, 128, DT]  (own slabs first within each tile),
             gids [P, (To+Tr)*128] int64 global node ids (for host
             pre-gather; padding points at node 0 with zero S rows))."""
    per_core = []
    order = np.argsort(dst, kind="stable")
    src, dst, val = src[order], dst[order], val[order]
    core_of = dst // NPC
    core_starts = np.searchsorted(core_of, np.arange(P + 1))
    for c in range(P):
        lo, hi = core_starts[c], core_starts[c + 1]
        s, d, v = src[lo:hi], dst[lo:hi] - c * NPC, val[lo:hi]
        tile_of = d // DT
        tile_starts = np.searchsorted(tile_of, np.arange(NDT + 1))
        groups = []
        for t in range(NDT):
            a, b = tile_starts[t], tile_starts[t + 1]
            st, dl, vt = s[a:b], d[a:b] - t * DT, v[a:b]
            uniq, inv = np.unique(st, return_inverse=True)
            S = np.zeros((max(len(uniq), 1), DT), np.float32)
            if len(uniq):
                np.add.at(S, (inv, dl), vt)
            else:
                uniq = np.zeros(1, np.int64)
            own = np.zeros(len(uniq), bool)  # own-split disabled (slower)
            perm = np.argsort(~own, kind="stable")
            groups.append((uniq[perm], S[perm], int(own.sum())))
        per_core.append(groups)

    def slabs(n):
        return (n + 127) // 128

    ETo = [max(slabs(per_core[c][t][2]) for c in range(P)) for t in range(NDT)]
    ETr = [max(max(slabs(len(per_core[c][t][0]) - per_core[c][t][2]), 1)
               for c in range(P)) for t in range(NDT)]
    To, Tr = sum(ETo), sum(ETr)
    offo = np.cumsum([0] + ETo[:-1])
    offr = np.cumsum([0] + ETr[:-1])
    offs = np.cumsum([0] + [a + b for a, b in zip(ETo, ETr)][:-1])
    idx_own = np.zeros((P, To, 128), np.int32)
    idx_rem = np.zeros((P, Tr, 128), np.int32)
    S_all = np.zeros((P, To + Tr, 128, DT), np.float32)
    gids = np.zeros((P, (To + Tr) * 128), np.int64)
    for c in range(P):
        for t in range(NDT):
            uniq, S, n_own = per_core[c][t]
            n = len(uniq)
            so = offs[t]
            # own slabs
            idx_own[c, offo[t]:offo[t] + slabs(n_own)].reshape(-1)[:n_own] = (
                uniq[:n_own] - c * NPC)
            S_all[c, so:so + ETo[t]].reshape(-1, DT)[:n_own] = S[:n_own]
            gids[c, so * 128:(so + ETo[t]) * 128][:n_own] = uniq[:n_own]
            # remote slabs
            n_rem = n - n_own
            sr = so + ETo[t]
            idx_rem[c, offr[t]:offr[t] + slabs(n_rem)].reshape(-1)[:n_rem] = (
                _to_padded_id(uniq[n_own:]))
            S_all[c, sr:sr + ETr[t]].reshape(-1, DT)[:n_rem] = S[n_own:]
            gids[c, sr * 128:(sr + ETr[t]) * 128][:n_rem] = uniq[n_own:]
    return (tuple(ETo), tuple(ETr), idx_own, idx_rem, S_all, gids)


def _idx_dev(idx_core):
    """[T, 128] int32 -> [128, T*8] int16 (wrap 16 partitions, replicate x8)."""
    flat = idx_core.reshape(-1)
    n = len(flat)
    a = np.zeros((16, n // 16), np.int16)
    a[np.arange(n) % 16, np.arange(n) // 16] = flat.astype(np.int16)
    return np.tile(a, (8, 1))


def _s_dev(S_core):
    """[T, 128, DT] -> [128, T*DT] bf16."""
    T = S_core.shape[0]
    return np.ascontiguousarray(
        S_core.transpose(1, 0, 2).reshape(128, T * DT)).astype(BF16)


def _w_dev(W):
    """[F, fo] -> [128, KC*fo] bf16 (chunk k at cols [k*fo, (k+1)*fo))."""
    fi, fo = W.shape
    k = fi // 128
    return np.ascontiguousarray(
        W.reshape(k, 128, fo).transpose(1, 0, 2).reshape(128, k * fo)).astype(BF16)


def _prep(x, edge_index, edge_weight, W1, b1, W2, b2, Wc, bc):
    row = np.asarray(edge_index[0], np.int64)
    col = np.asarray(edge_index[1], np.int64)
    w = np.asarray(edge_weight, np.float32)

    # GCN norm (layers 1 & 2): deg over dest (col) + 1 self loop.
    deg = np.zeros(N, np.float32)
    np.add.at(deg, col, w)
    deg += 1.0
    dinv = (1.0 / np.sqrt(deg)).astype(np.float32)
    g_src = np.concatenate([row, np.arange(N)])
    g_dst = np.concatenate([col, np.arange(N)])
    g_val = np.concatenate([dinv[row] * w * dinv[col], dinv * dinv]).astype(np.float32)

    # Cheb: drop self loops, deg over src (row), negate (lhat = -A_norm).
    keep = row != col
    r0, c0, w0 = row[keep], col[keep], w[keep]
    deg2 = np.zeros(N, np.float32)
    np.add.at(deg2, r0, w0)
    dinv2 = np.where(deg2 > 0, 1.0 / np.sqrt(deg2), 0.0).astype(np.float32)
    c_val = -(dinv2[r0] * w0 * dinv2[c0]).astype(np.float32)

    ETgo, ETgr, idxgo, idxgr, Sg, gidsg = _build_edge_tiles(g_src, g_dst, g_val)
    ETco, ETcr, idxco, idxcr, Sc, _ = _build_edge_tiles(r0, c0, c_val)

    x = np.asarray(x, np.float32)
    xr = x.astype(BF16)

    Wc = np.asarray(Wc, np.float32)
    com = dict(
        w1=_w_dev(np.asarray(W1, np.float32)),
        w2=_w_dev(np.asarray(W2, np.float32)),
        wa=_w_dev(Wc[0] - Wc[2]),
        wb=_w_dev(Wc[1]),
        wc2=_w_dev(2.0 * Wc[2]),
        ident=np.eye(128, dtype=BF16),
        ident8=np.eye(128, dtype=BF16),
    )
    biases = (np.asarray(b1, np.float32), np.asarray(b2, np.float32),
              np.asarray(bc, np.float32))
    in_maps = []
    for c in range(P):
        m = dict(com)
        # Layer 1 gathers from x (host-known): pre-gather the message rows
        # so the device streams them with static DMA (no descriptor gen).
        m["msgs1"] = np.ascontiguousarray(xr[gidsg[c]])
        if idxgo.shape[1]:
            m["idxgo"] = _idx_dev(idxgo[c])
        m["idxgr"] = _idx_dev(idxgr[c])
        m["sg"] = _s_dev(Sg[c])
        if idxco.shape[1]:
            m["idxco"] = _idx_dev(idxco[c])
        m["idxcr"] = _idx_dev(idxcr[c])
        m["sc"] = _s_dev(Sc[c])
        in_maps.append(m)
    return (ETgo, ETgr, ETco, ETcr), biases, in_maps


# ------------------------------------------------------------- bass program

_CACHE = {}


def _build_program(ETs, has_bias):
    import os
    ETgo, ETgr, ETco, ETcr = ETs
    key = (ETs, has_bias, os.environ.get("GNN_PHASES", "9"))
    if key in _CACHE:
        return _CACHE[key]
    TGo, TGr = sum(ETgo), sum(ETgr)
    TCo, TCr = sum(ETco), sum(ETcr)
    TTg, TTc = TGo + TGr, TCo + TCr
    ETg = [a + b for a, b in zip(ETgo, ETgr)]
    ETc = [a + b for a, b in zip(ETco, ETcr)]
    ETMAX = max(max(ETg), max(ETc))
    ETCMAX = max(ETc)

    nc = bacc.Bacc("TRN2", target_bir_lowering=False, num_devices=P,
                   num_swdge_queues=4)
    msgs1 = nc.dram_tensor("msgs1", [TTg * 128, F], BF16D, kind="ExternalInput")
    idxgo = (nc.dram_tensor("idxgo", [128, TGo * 8], I16, kind="ExternalInput")
             if TGo else None)
    idxgr = nc.dram_tensor("idxgr", [128, TGr * 8], I16, kind="ExternalInput")
    sg = nc.dram_tensor("sg", [128, TTg * DT], BF16D, kind="ExternalInput")
    idxco = (nc.dram_tensor("idxco", [128, TCo * 8], I16, kind="ExternalInput")
             if TCo else None)
    idxcr = nc.dram_tensor("idxcr", [128, TCr * 8], I16, kind="ExternalInput")
    sc = nc.dram_tensor("sc", [128, TTc * DT], BF16D, kind="ExternalInput")
    w1 = nc.dram_tensor("w1", [128, KC * F], BF16D, kind="ExternalInput")
    w2 = nc.dram_tensor("w2", [128, KC * F], BF16D, kind="ExternalInput")
    wa = nc.dram_tensor("wa", [128, KC * DOUT], BF16D, kind="ExternalInput")
    wb = nc.dram_tensor("wb", [128, KC * DOUT], BF16D, kind="ExternalInput")
    wc2 = nc.dram_tensor("wc2", [128, KC * DOUT], BF16D, kind="ExternalInput")
    ident = nc.dram_tensor("ident", [128, 128], BF16D, kind="ExternalInput")
    ident8 = nc.dram_tensor("ident8", [128, 128], BF16D, kind="ExternalInput")
    if has_bias:
        brows = nc.dram_tensor("brows", [1, 2 * F + DOUT], FP32, kind="ExternalInput")
    outp = nc.dram_tensor("out", [NPAD, DOUT], FP32, kind="ExternalOutput")

    h1c = nc.dram_tensor("h1c", [NPAD, F], BF16D, kind="Internal")
    h1f = nc.dram_tensor("h1f", [NTOT, F], BF16D, kind="Internal", addr_space="Shared")
    pqc = nc.dram_tensor("pqc", [NPAD, F], FP8D, kind="Internal")
    pqf = nc.dram_tensor("pqf", [NTOT, F], FP8D, kind="Internal", addr_space="Shared")
    rc = nc.dram_tensor("rc", [NPAD, DOUT], FP8D, kind="Internal")
    rf = nc.dram_tensor("rf", [NTOT, DOUT], FP8D, kind="Internal",
                        addr_space="Shared")

    Exp = mybir.ActivationFunctionType.Exp
    Alu = mybir.AluOpType

    with tile.TileContext(nc) as tc:
        with (
            tc.tile_pool(name="const", bufs=1) as cpool,
            tc.tile_pool(name="keep", bufs=1) as kpool,
            tc.tile_pool(name="msgs", bufs=2) as mpool,
            tc.tile_pool(name="msgsB", bufs=6) as mpoolB,
            tc.tile_pool(name="work", bufs=3) as wpool,
            tc.tile_pool(name="psum", bufs=2, space="PSUM") as ppool,
            tc.tile_pool(name="psum3", bufs=4, space="PSUM") as ppool3,
        ):
            lib = nc.gpsimd.load_library(library_config.mlp)

            id_sb = cpool.tile([128, 128], BF16D, tag="id")
            nc.sync.dma_start(id_sb[:], ident[:])
            id8_sb = cpool.tile([128, 128], BF16D, tag="id8")
            nc.sync.dma_start(id8_sb[:], ident8[:])

            igo_sb = (cpool.tile([128, TGo * 8], I16, tag="igo")
                      if TGo else None)
            igr_sb = cpool.tile([128, TGr * 8], I16, tag="igr")
            ico_sb = (cpool.tile([128, TCo * 8], I16, tag="ico")
                      if TCo else None)
            icr_sb = cpool.tile([128, TCr * 8], I16, tag="icr")
            offgo = np.cumsum([0] + list(ETgo[:-1]))
            offgr = np.cumsum([0] + list(ETgr[:-1]))
            offgs = np.cumsum([0] + ETg[:-1])
            offco = np.cumsum([0] + list(ETco[:-1]))
            offcr = np.cumsum([0] + list(ETcr[:-1]))
            offcs = np.cumsum([0] + [a + b for a, b in zip(ETco, ETcr)][:-1])
            # L1 needs only w1 + the GCN S tiles; everything else loads later
            # (after the L1 streams are underway) so L1 starts immediately.
            w1_sb = cpool.tile([128, KC * F], BF16D, tag="w1")
            nc.sync.dma_start(w1_sb[:], w1[:])
            if has_bias:
                br_sb = cpool.tile([1, 2 * F + DOUT], FP32, tag="br")
                nc.sync.dma_start(br_sb[:], brows[:])
                ones_sb = cpool.tile([1, 128], FP32, tag="ones")
                nc.vector.memset(ones_sb[:], 1.0)

            w2_sb = cpool.tile([128, KC * F], BF16D, tag="w2")
            wa_sb = cpool.tile([128, KC * DOUT], BF16D, tag="wa")
            wb_sb = cpool.tile([128, KC * DOUT], BF16D, tag="wb")
            wc2_sb = cpool.tile([128, KC * DOUT], BF16D, tag="wc2")

            def late_loads():
                """Small load steps, drained a couple per L1 tile so they
                interleave with (rather than block) the msgs1 streams."""
                for t in range(4, NDT):
                    a, b = offgs[t] * DT, (offgs[t] + ETg[t]) * DT
                    yield lambda a=a, b=b: nc.sync.dma_start(
                        sg_sb[:, a:b], sg[:, a:b])
                yield lambda: nc.sync.dma_start(w2_sb[:], w2[:])
                if TGo:
                    yield lambda: nc.sync.dma_start(igo_sb[:], idxgo[:])
                yield lambda: nc.sync.dma_start(igr_sb[:], idxgr[:])
                yield lambda: nc.sync.dma_start(wa_sb[:], wa[:])
                yield lambda: nc.sync.dma_start(wb_sb[:], wb[:])
                yield lambda: nc.sync.dma_start(wc2_sb[:], wc2[:])
                if TCo:
                    yield lambda: nc.sync.dma_start(ico_sb[:], idxco[:])
                yield lambda: nc.sync.dma_start(icr_sb[:], idxcr[:])

            h2keep = kpool.tile([128, NDT, F], BF16D, tag="h2k")
            lapkeep = kpool.tile([128, NDT, DOUT], BF16D, tag="lap")

            first_gather = [0]
            qctr = [0]

            def scatter(src_local, src_full, eto, etr, io, ir, so,
                        idxo_sb, idxr_sb, s_sb, width=F,
                        msgs_tag="msgs", msgs_w=None, msgs_dt=BF16D):
                """One dest tile: gather own slabs from the Local staging
                tensor (no AllGather dependency — flows into the AG window)
                and remote slabs from the AllGathered table, into one msgs
                tile; one psum accumulation over [own | remote] S slabs."""
                mw = msgs_w if msgs_w is not None else ETMAX
                pool = mpool if msgs_tag == "msgs" else mpoolB
                msgs = pool.tile([128, mw, width], msgs_dt, tag=msgs_tag)

                def gathers(src, idx_sb, base, lo, n, nq, own):
                    bounds = [lo + n * i // nq for i in range(nq + 1)]
                    for a, b in zip(bounds[:-1], bounds[1:]):
                        if b <= a:
                            continue
                        # Queue 3 is reserved for own-slab gathers (no
                        # AllGather dependency) so they are never stuck
                        # behind a remote gather blocked at a queue head
                        # waiting for the collective.
                        if own:
                            q = 3
                        else:
                            q = qctr[0] % (3 if eto else 4)
                            qctr[0] += 1
                        gi = nc.gpsimd.dma_gather(
                            msgs[:, a:b, :], src[:],
                            idx_sb[:, (base + a - lo) * 8:
                                   (base + b - lo) * 8],
                            (b - a) * 128, (b - a) * 128, width,
                            single_packet=False, queue_num=q)
                        if first_gather[0] < 4:
                            add_dep_helper(gi.ins, lib.ins,
                                           reason="mlp lib before gather")
                            first_gather[0] += 1

                gathers(src_local, idxo_sb, io, 0, eto, 1, True)
                gathers(src_full, idxr_sb, ir, eto, etr,
                        min(3 if eto else 4, etr), False)
                # S slab as stationary lhsT, msgs streamed as rhs:
                # ps[d, f] += S[e, d].T @ msgs[e, f]  (node-major aggregate).
                ps = ppool3.tile([128, F], FP32, tag="psT")
                et = eto + etr
                for g in range(et):
                    nc.tensor.matmul(
                        ps[:, :width],
                        s_sb[:, (so + g) * DT:(so + g + 1) * DT],
                        msgs[:, g, :],
                        start=(g == 0),
                        stop=(g == et - 1))
                return ps

            def celu(z_ps, width, out_ap):
                """out = max(z,0) + min(exp(z)-1, 0); z read from PSUM."""
                e = wpool.tile([128, F], FP32, tag="e")
                nc.scalar.activation(e[:, :width], z_ps, Exp)
                nc.vector.tensor_scalar(
                    e[:, :width], e[:, :width], 1.0, 0.0,
                    Alu.subtract, Alu.min)
                nc.vector.scalar_tensor_tensor(
                    out_ap, z_ps, 0.0, e[:, :width], Alu.max, Alu.add)

            def gemm_bias(z_ps, width, b_off, stop=False):
                if has_bias:
                    nc.tensor.matmul(
                        z_ps, ones_sb[:],
                        br_sb[:, b_off:b_off + width],
                        start=False, stop=stop)

            def allgather(cin, cout):
                nc.gpsimd.collective_compute(
                    "AllGather", Alu.bypass,
                    replica_groups=[list(range(P))],
                    ins=[cin[:]],
                    outs=[cout[:]])

            import os
            PH = int(os.environ.get("GNN_PHASES", "9"))

            sgp_cm = tc.tile_pool(name="sgp", bufs=1)
            sgp = sgp_cm.__enter__()
            sg_sb = sgp.tile([128, TTg * DT], BF16D, tag="sg")
            # Only the first few S tiles load upfront; the rest drip through
            # late_loads so tile 0's msgs1 stream isn't queued behind 5.5MB
            # of S-table DMA on the sync engine.
            for t in range(4):
                a, b = offgs[t] * DT, (offgs[t] + ETg[t]) * DT
                nc.sync.dma_start(sg_sb[:, a:b], sg[:, a:b])

            def stream_scatter(t):
                """L1 variant of scatter: stream host-pregathered x rows
                (contiguous, static HWDGE DMA on two queues) + S matmuls."""
                from concourse.ap import AP as _AP
                o = offgs[t]
                et = ETg[t]
                msgs = mpool.tile([128, ETMAX, F], BF16D, tag="msgs")
                ha = (et + 1) // 2
                for eng, a, b in ((nc.sync, 0, ha), (nc.scalar, ha, et)):
                    if b <= a:
                        continue
                    eng.dma_start(
                        msgs[:, a:b, :],
                        _AP(msgs1, int(o + a) * 128 * F,
                            [[F, 128], [128 * F, b - a], [1, F]]))
                pst = ppool3.tile([128, F], FP32, tag="psT")
                for g in range(et):
                    nc.tensor.matmul(
                        pst[:],
                        sg_sb[:, (o + g) * DT:(o + g + 1) * DT],
                        msgs[:, g, :],
                        start=(g == 0), stop=(g == et - 1))
                return pst

            def gcn_tail(ps, w_sb, b_off):
                """agg -> transpose -> GEMM -> bias; returns psum z [128,F]."""
                agg = wpool.tile([128, F], BF16D, tag="agg")
                nc.vector.tensor_copy(agg[:], ps[:])
                tps = ppool.tile([128, KC, 128], BF16D, tag="tps")
                for k in range(KC):
                    nc.tensor.transpose(
                        tps[:, k, :], agg[:, k * 128:(k + 1) * 128], id_sb[:])
                aggT = wpool.tile([128, KC, 128], BF16D, tag="aggT")
                nc.vector.tensor_copy(aggT[:], tps[:])
                z = ppool.tile([128, F], FP32, tag="z")
                for k in range(KC):
                    nc.tensor.matmul(
                        z[:], aggT[:, k, :], w_sb[:, k * F:(k + 1) * F],
                        start=(k == 0), stop=False)
                gemm_bias(z[:], F, b_off, stop=True)
                return z

            # ---- layer 1: h1 = celu((Ag @ x) @ W1 + b1)
            _late = late_loads()
            for t in range(NDT):
                ps = stream_scatter(t)
                z = gcn_tail(ps, w1_sb, 0)
                h = wpool.tile([128, F], BF16D, tag="h")
                celu(z[:], F, h[:])
                nc.sync.dma_start(h1c[t * 128:(t + 1) * 128, :], h[:])
                if t >= 1:
                    for _ in range(3):
                        step = next(_late, None)
                        if step is not None:
                            step()
            for step in _late:
                step()
            if PH >= 2:
                allgather(h1c, h1f)

            # ---- layer 2: h2 = celu((Ag @ h1) @ W2 + b2), kept on chip.
            #      Per tile, also transpose h2 to feature-major and project
            #      pq = h2 @ [Wb | Wc2] for the cheb passes.
            if PH >= 3:
                for t in range(NDT):
                    ps = scatter(h1c, h1f, ETgo[t], ETgr[t], int(offgo[t]),
                                 int(offgr[t]), int(offgs[t]),
                                 igo_sb, igr_sb, sg_sb)
                    z = gcn_tail(ps, w2_sb, F)
                    h2t = wpool.tile([128, F], BF16D, tag="h")
                    celu(z[:], F, h2t[:])
                    # feature-major h2 for the three h2 @ W GEMM terms
                    tps2 = ppool.tile([128, KC, 128], BF16D, tag="tps")
                    for k in range(KC):
                        nc.tensor.transpose(
                            tps2[:, k, :], h2t[:, k * 128:(k + 1) * 128],
                            id_sb[:])
                    nc.vector.tensor_copy(h2keep[:, t, :], tps2[:])
                    if PH >= 4:
                        # pq = h2 @ [Wb | Wc2]  (node-major in psum)
                        zpq = ppool.tile([128, F], FP32, tag="z")
                        for k in range(KC):
                            nc.tensor.matmul(
                                zpq[:, 0:DOUT],
                                h2keep[:, t, k * 128:(k + 1) * 128],
                                wb_sb[:, k * DOUT:(k + 1) * DOUT],
                                start=(k == 0), stop=(k == KC - 1))
                        for k in range(KC):
                            nc.tensor.matmul(
                                zpq[:, DOUT:2 * DOUT],
                                h2keep[:, t, k * 128:(k + 1) * 128],
                                wc2_sb[:, k * DOUT:(k + 1) * DOUT],
                                start=(k == 0), stop=(k == KC - 1))
                        pq = wpool.tile([128, F], FP8D, tag="h8")
                        nc.vector.tensor_copy(pq[:], zpq[:])
                        nc.sync.dma_start(pqc[t * 128:(t + 1) * 128, :], pq[:])
                if PH >= 4:
                    allgather(pqc, pqf)

            # sg is dead after L2 -- release its SBUF zone and put the cheb S
            # tiles there (loads overlap the pq AllGather window).
            sgp_cm.__exit__(None, None, None)
            scp_cm = tc.tile_pool(name="scp", bufs=1)
            scp = scp_cm.__enter__()
            sc_sb = scp.tile([128, TTc * DT], BF16D, tag="sc")
            for t in range(NDT):
                a = offcs[t] * DT
                b = (offcs[t] + ETco[t] + ETcr[t]) * DT
                nc.sync.dma_start(sc_sb[:, a:b], sc[:, a:b])
            # ---- cheb pass A: [lhat_p | lhat_q] = lhat(pq); keep lhat_p,
            #      AllGather r = lhat_q (256 wide).
            if PH >= 5:
                for t in range(NDT):
                    ps = scatter(pqc, pqf, ETco[t], ETcr[t], int(offco[t]),
                                 int(offcr[t]), int(offcs[t]),
                                 ico_sb, icr_sb, sc_sb,
                                 msgs_tag="msgs256", msgs_w=ETMAX,
                                 msgs_dt=FP8D)
                    nc.vector.tensor_copy(lapkeep[:, t, :], ps[:, 0:DOUT])
                    rt = wpool.tile([128, DOUT], FP8D, tag="rt")
                    nc.vector.tensor_copy(rt[:], ps[:, DOUT:2 * DOUT])
                    nc.sync.dma_start(rc[t * 128:(t + 1) * 128, :], rt[:])
                allgather(rc, rf)

            # ---- cheb pass B + output:
            # out = celu(lhat(r) + h2 @ Wa + lhat_p + bc)
            if PH >= 6:
                # Local terms zoL = h2 @ Wa + lhat_p + bc precomputed on PE
                # while the r AllGather is in flight.
                zokeep = kpool.tile([128, NDT, DOUT], FP32, tag="zok")
                for t in range(NDT):
                    zl = ppool.tile([128, F], FP32, tag="z")
                    zv = zl[:, :DOUT]
                    for k in range(KC):
                        nc.tensor.matmul(
                            zv, h2keep[:, t, k * 128:(k + 1) * 128],
                            wa_sb[:, k * DOUT:(k + 1) * DOUT],
                            start=(k == 0), stop=False)
                    nc.tensor.matmul(
                        zv, id8_sb[:], lapkeep[:, t, :],
                        start=False, stop=not has_bias)
                    gemm_bias(zv, DOUT, 2 * F, stop=True)
                    nc.vector.tensor_copy(zokeep[:, t, :], zv)
                for t in range(NDT):
                    ps = scatter(rc, rf, ETco[t], ETcr[t], int(offco[t]),
                                 int(offcr[t]), int(offcs[t]),
                                 ico_sb, icr_sb, sc_sb,
                                 width=DOUT, msgs_tag="msgs256",
                                 msgs_w=ETCMAX, msgs_dt=FP8D)
                    zs = wpool.tile([128, DOUT], FP32, tag="zs")
                    nc.vector.scalar_tensor_tensor(
                        zs[:], ps[:, :DOUT], 1.0, zokeep[:, t, :],
                        Alu.mult, Alu.add)
                    of = wpool.tile([128, DOUT], FP32, tag="of")
                    celu(zs[:], DOUT, of[:])
                    nc.sync.dma_start(outp[t * 128:(t + 1) * 128, :], of[:])

            scp_cm.__exit__(None, None, None)

    nc.compile()
    _CACHE[key] = nc
    return nc


# ------------------------------------------------------------------- driver

def _run(inputs, trace=False, tmpdir=None):
    ETs, biases, in_maps = _prep(**inputs)
    has_bias = any(np.any(b != 0) for b in biases)
    if has_bias:
        brow = np.concatenate(biases).astype(np.float32)[None, :]
        for m in in_maps:
            m["brows"] = brow
    nc = _build_program(ETs, has_bias)
    res = run_bass_kernel_spmd(nc, in_maps, core_ids=list(range(P)),
                               trace=trace, tmpdir=tmpdir)
    out = np.concatenate(
        [res.results[c]["out"][:NPC] for c in range(P)], axis=0)
    return out.astype(np.float32), res


def kernel(**inputs) -> np.ndarray:
    out, _ = _run(inputs)
    return out


# revision 48
# speedup vs baseline: 1.1014x; 1.1014x over previous
"""Trainium2 Bass kernel for the ChebConv GNN problem
(nn_ChebConvConvolutional): 2x GCNConv + 1x ChebConv(K=3), N=10000 nodes,
E=160000 edges, F=512, celu activations.

Strategy (8 NeuronCores, SPMD):
  * Nodes are sharded 1250/core (padded to 1280). Edges are sharded by
    destination core and grouped into 128-dest tiles; per dest-tile the
    source nodes are deduplicated and the edge weights are baked into dense
    [128 src x 128 dst] one-hot "S" matrices (GCN self-loops folded in as
    edges with value dinv^2, Cheb normalization negated so the scatter
    directly produces lhat).
  * Every graph op is computed aggregate-first: h = celu((A @ x) @ W + b);
    layers end with a small AllGather of the core's 1280-row bf16 slice.
  * Layer 1's gather indices are host-known, so its message rows are
    pre-gathered on the host and streamed with static HWDGE DMA (no
    SWDGE descriptor generation). Later passes dma_gather from the
    AllGathered tables; the tensor engine computes
    ps[d, f] += S[e, d].T @ msgs[e, f], then dense GEMMs, and
    celu = max(z,0) + min(exp(z)-1, 0) runs on ACT + DVE.
  * SBUF is time-shared via scoped tile pools: the GCN S tiles release
    after layer 2 and the cheb S tiles load into the freed zone during
    the pq AllGather; message pools are triple/quadruple buffered.
  * ChebConv K=3 uses the commuted form (lhat is row-mixing, W col-mixing):
        out = celu(h2 @ Wa + lhat(h2 @ Wb) + lhat(lhat(h2 @ Wc2)) + bc)
    with Wa = Wk0 - Wk2, Wb = Wk1, Wc2 = 2*Wk2. After layer 2 each core
    projects its h2 tiles to pq = h2 @ [Wb | Wc2] (512 wide), AllGathers pq
    once; pass A scatters pq into [lhat_p | lhat_q], keeps lhat_p on chip
    and AllGathers r = lhat_q (256 wide); pass B scatters r (half-width
    gather) and finishes out = celu(psum + h2 @ Wa + lhat_p + bc) in psum.
"""
import numpy as np
import ml_dtypes

import concourse.bacc as bacc
import concourse.mybir as mybir
import concourse.tile as tile
from concourse import library_config
from concourse.bass_utils import run_bass_kernel_spmd
from concourse.tile import add_dep_helper

BF16 = ml_dtypes.bfloat16
FP32 = mybir.dt.float32
BF16D = mybir.dt.bfloat16
FP8D = mybir.dt.float8e4
I16 = mybir.dt.int16

P = 8            # cores
N = 10000        # nodes
NPC = N // P     # nodes per core
NPAD = 1280      # padded nodes per core
NTOT = NPAD * P
F = 512          # feature width of x / h1 / h2
DOUT = 256
DT = 128         # dests per dest tile
NDT = NPAD // DT # dest tiles per core
KC = F // 128    # contraction chunks (4)


# ----------------------------------------------------------------- host prep

def _to_padded_id(n):
    """Global node id -> row in the AllGather global layout: [P ranks][NPAD]."""
    r = n // NPC
    return r * NPAD + (n % NPC)


def _build_edge_tiles(src, dst, val):
    """Shard by dest core, tile by 128 dests, dedup sources per tile.
    Own-rank sources are ordered first so their slabs can be aggregated
    from the Local staging tensor during the AllGather window.

    Returns (ETo, ETr: per-tile own/remote slab counts,
             idx_own [P, To, 128] int32 LOCAL row ids,
             idx_rem [P, Tr, 128] int32 padded global ids,
             S [P, To+Tr, 128, DT]  (own slabs first within each tile),
             gids [P, (To+Tr)*128] int64 global node ids (for host
             pre-gather; padding points at node 0 with zero S rows))."""
    per_core = []
    order = np.argsort(dst, kind="stable")
    src, dst, val = src[order], dst[order], val[order]
    core_of = dst // NPC
    core_starts = np.searchsorted(core_of, np.arange(P + 1))
    for c in range(P):
        lo, hi = core_starts[c], core_starts[c + 1]
        s, d, v = src[lo:hi], dst[lo:hi] - c * NPC, val[lo:hi]
        tile_of = d // DT
        tile_starts = np.searchsorted(tile_of, np.arange(NDT + 1))
        groups = []
        for t in range(NDT):
            a, b = tile_starts[t], tile_starts[t + 1]
            st, dl, vt = s[a:b], d[a:b] - t * DT, v[a:b]
            uniq, inv = np.unique(st, return_inverse=True)
            S = np.zeros((max(len(uniq), 1), DT), np.float32)
            if len(uniq):
                np.add.at(S, (inv, dl), vt)
            else:
                uniq = np.zeros(1, np.int64)
            own = np.zeros(len(uniq), bool)  # own-split disabled (slower)
            perm = np.argsort(~own, kind="stable")
            groups.append((uniq[perm], S[perm], int(own.sum())))
        per_core.append(groups)

    def slabs(n):
        return (n + 127) // 128

    ETo = [max(slabs(per_core[c][t][2]) for c in range(P)) for t in range(NDT)]
    ETr = [max(max(slabs(len(per_core[c][t][0]) - per_core[c][t][2]), 1)
               for c in range(P)) for t in range(NDT)]
    To, Tr = sum(ETo), sum(ETr)
    offo = np.cumsum([0] + ETo[:-1])
    offr = np.cumsum([0] + ETr[:-1])
    offs = np.cumsum([0] + [a + b for a, b in zip(ETo, ETr)][:-1])
    idx_own = np.zeros((P, To, 128), np.int32)
    idx_rem = np.zeros((P, Tr, 128), np.int32)
    S_all = np.zeros((P, To + Tr, 128, DT), np.float32)
    gids = np.zeros((P, (To + Tr) * 128), np.int64)
    for c in range(P):
        for t in range(NDT):
            uniq, S, n_own = per_core[c][t]
            n = len(uniq)
            so = offs[t]
            # own slabs
            idx_own[c, offo[t]:offo[t] + slabs(n_own)].reshape(-1)[:n_own] = (
                uniq[:n_own] - c * NPC)
            S_all[c, so:so + ETo[t]].reshape(-1, DT)[:n_own] = S[:n_own]
            gids[c, so * 128:(so + ETo[t]) * 128][:n_own] = uniq[:n_own]
            # remote slabs
            n_rem = n - n_own
            sr = so + ETo[t]
            idx_rem[c, offr[t]:offr[t] + slabs(n_rem)].reshape(-1)[:n_rem] = (
                _to_padded_id(uniq[n_own:]))
            S_all[c, sr:sr + ETr[t]].reshape(-1, DT)[:n_rem] = S[n_own:]
            gids[c, sr * 128:(sr + ETr[t]) * 128][:n_rem] = uniq[n_own:]
    return (tuple(ETo), tuple(ETr), idx_own, idx_rem, S_all, gids)


def _idx_dev(idx_core):
    """[T, 128] int32 -> [128, T*8] int16 (wrap 16 partitions, replicate x8)."""
    flat = idx_core.reshape(-1)
    n = len(flat)
    a = np.zeros((16, n // 16), np.int16)
    a[np.arange(n) % 16, np.arange(n) // 16] = flat.astype(np.int16)
    return np.tile(a, (8, 1))


def _s_dev(S_core):
    """[T, 128, DT] -> [128, T*DT] bf16."""
    T = S_core.shape[0]
    return np.ascontiguousarray(
        S_core.transpose(1, 0, 2).reshape(128, T * DT)).astype(BF16)


def _w_dev(W):
    """[F, fo] -> [128, KC*fo] bf16 (chunk k at cols [k*fo, (k+1)*fo))."""
    fi, fo = W.shape
    k = fi // 128
    return np.ascontiguousarray(
        W.reshape(k, 128, fo).transpose(1, 0, 2).reshape(128, k * fo)).astype(BF16)


def _prep(x, edge_index, edge_weight, W1, b1, W2, b2, Wc, bc):
    row = np.asarray(edge_index[0], np.int64)
    col = np.asarray(edge_index[1], np.int64)
    w = np.asarray(edge_weight, np.float32)

    # GCN norm (layers 1 & 2): deg over dest (col) + 1 self loop.
    deg = np.zeros(N, np.float32)
    np.add.at(deg, col, w)
    deg += 1.0
    dinv = (1.0 / np.sqrt(deg)).astype(np.float32)
    g_src = np.concatenate([row, np.arange(N)])
    g_dst = np.concatenate([col, np.arange(N)])
    g_val = np.concatenate([dinv[row] * w * dinv[col], dinv * dinv]).astype(np.float32)

    # Cheb: drop self loops, deg over src (row), negate (lhat = -A_norm).
    keep = row != col
    r0, c0, w0 = row[keep], col[keep], w[keep]
    deg2 = np.zeros(N, np.float32)
    np.add.at(deg2, r0, w0)
    dinv2 = np.where(deg2 > 0, 1.0 / np.sqrt(deg2), 0.0).astype(np.float32)
    c_val = -(dinv2[r0] * w0 * dinv2[c0]).astype(np.float32)

    ETgo, ETgr, idxgo, idxgr, Sg, gidsg = _build_edge_tiles(g_src, g_dst, g_val)
    ETco, ETcr, idxco, idxcr, Sc, _ = _build_edge_tiles(r0, c0, c_val)

    x = np.asarray(x, np.float32)
    xr = x.astype(BF16)

    Wc = np.asarray(Wc, np.float32)
    com = dict(
        w1=_w_dev(np.asarray(W1, np.float32)),
        w2=_w_dev(np.asarray(W2, np.float32)),
        wa=_w_dev(Wc[0] - Wc[2]),
        wb=_w_dev(Wc[1]),
        wc2=_w_dev(2.0 * Wc[2]),
        ident=np.eye(128, dtype=BF16),
        ident8=np.eye(128, dtype=BF16),
    )
    biases = (np.asarray(b1, np.float32), np.asarray(b2, np.float32),
              np.asarray(bc, np.float32))
    in_maps = []
    for c in range(P):
        m = dict(com)
        # Layer 1 gathers from x (host-known): pre-gather the message rows
        # so the device streams them with static DMA (no descriptor gen).
        m["msgs1"] = np.ascontiguousarray(xr[gidsg[c]])
        if idxgo.shape[1]:
            m["idxgo"] = _idx_dev(idxgo[c])
        m["idxgr"] = _idx_dev(idxgr[c])
        m["sg"] = _s_dev(Sg[c])
        if idxco.shape[1]:
            m["idxco"] = _idx_dev(idxco[c])
        m["idxcr"] = _idx_dev(idxcr[c])
        m["sc"] = _s_dev(Sc[c])
        in_maps.append(m)
    return (ETgo, ETgr, ETco, ETcr), biases, in_maps


# ------------------------------------------------------------- bass program

_CACHE = {}


def _build_program(ETs, has_bias):
    import os
    ETgo, ETgr, ETco, ETcr = ETs
    key = (ETs, has_bias, os.environ.get("GNN_PHASES", "9"))
    if key in _CACHE:
        return _CACHE[key]
    TGo, TGr = sum(ETgo), sum(ETgr)
    TCo, TCr = sum(ETco), sum(ETcr)
    TTg, TTc = TGo + TGr, TCo + TCr
    ETg = [a + b for a, b in zip(ETgo, ETgr)]
    ETc = [a + b for a, b in zip(ETco, ETcr)]
    ETMAX = max(max(ETg), max(ETc))
    ETCMAX = max(ETc)

    nc = bacc.Bacc("TRN2", target_bir_lowering=False, num_devices=P,
                   num_swdge_queues=4)
    msgs1 = nc.dram_tensor("msgs1", [TTg * 128, F], BF16D, kind="ExternalInput")
    idxgo = (nc.dram_tensor("idxgo", [128, TGo * 8], I16, kind="ExternalInput")
             if TGo else None)
    idxgr = nc.dram_tensor("idxgr", [128, TGr * 8], I16, kind="ExternalInput")
    sg = nc.dram_tensor("sg", [128, TTg * DT], BF16D, kind="ExternalInput")
    idxco = (nc.dram_tensor("idxco", [128, TCo * 8], I16, kind="ExternalInput")
             if TCo else None)
    idxcr = nc.dram_tensor("idxcr", [128, TCr * 8], I16, kind="ExternalInput")
    sc = nc.dram_tensor("sc", [128, TTc * DT], BF16D, kind="ExternalInput")
    w1 = nc.dram_tensor("w1", [128, KC * F], BF16D, kind="ExternalInput")
    w2 = nc.dram_tensor("w2", [128, KC * F], BF16D, kind="ExternalInput")
    wa = nc.dram_tensor("wa", [128, KC * DOUT], BF16D, kind="ExternalInput")
    wb = nc.dram_tensor("wb", [128, KC * DOUT], BF16D, kind="ExternalInput")
    wc2 = nc.dram_tensor("wc2", [128, KC * DOUT], BF16D, kind="ExternalInput")
    ident = nc.dram_tensor("ident", [128, 128], BF16D, kind="ExternalInput")
    ident8 = nc.dram_tensor("ident8", [128, 128], BF16D, kind="ExternalInput")
    if has_bias:
        brows = nc.dram_tensor("brows", [1, 2 * F + DOUT], FP32, kind="ExternalInput")
    outp = nc.dram_tensor("out", [NPAD, DOUT], FP32, kind="ExternalOutput")

    h1c = nc.dram_tensor("h1c", [NPAD, F], BF16D, kind="Internal")
    h1f = nc.dram_tensor("h1f", [NTOT, F], BF16D, kind="Internal", addr_space="Shared")
    pqc = nc.dram_tensor("pqc", [NPAD, F], FP8D, kind="Internal")
    pqf = nc.dram_tensor("pqf", [NTOT, F], FP8D, kind="Internal", addr_space="Shared")
    rc = nc.dram_tensor("rc", [NPAD, DOUT], FP8D, kind="Internal")
    rf = nc.dram_tensor("rf", [NTOT, DOUT], FP8D, kind="Internal",
                        addr_space="Shared")

    Exp = mybir.ActivationFunctionType.Exp
    Alu = mybir.AluOpType

    with tile.TileContext(nc) as tc:
        with (
            tc.tile_pool(name="const", bufs=1) as cpool,
            tc.tile_pool(name="keep", bufs=1) as kpool,
            tc.tile_pool(name="msgs", bufs=2) as mpool,
            tc.tile_pool(name="msgsB", bufs=6) as mpoolB,
            tc.tile_pool(name="work", bufs=3) as wpool,
            tc.tile_pool(name="psum", bufs=2, space="PSUM") as ppool,
            tc.tile_pool(name="psum3", bufs=4, space="PSUM") as ppool3,
        ):
            lib = nc.gpsimd.load_library(library_config.mlp)

            id_sb = cpool.tile([128, 128], BF16D, tag="id")
            nc.sync.dma_start(id_sb[:], ident[:])
            id8_sb = cpool.tile([128, 128], BF16D, tag="id8")
            nc.sync.dma_start(id8_sb[:], ident8[:])

            igo_sb = (cpool.tile([128, TGo * 8], I16, tag="igo")
                      if TGo else None)
            igr_sb = cpool.tile([128, TGr * 8], I16, tag="igr")
            ico_sb = (cpool.tile([128, TCo * 8], I16, tag="ico")
                      if TCo else None)
            icr_sb = cpool.tile([128, TCr * 8], I16, tag="icr")
            offgo = np.cumsum([0] + list(ETgo[:-1]))
            offgr = np.cumsum([0] + list(ETgr[:-1]))
            offgs = np.cumsum([0] + ETg[:-1])
            offco = np.cumsum([0] + list(ETco[:-1]))
            offcr = np.cumsum([0] + list(ETcr[:-1]))
            offcs = np.cumsum([0] + [a + b for a, b in zip(ETco, ETcr)][:-1])
            # L1 needs only w1 + the GCN S tiles; everything else loads later
            # (after the L1 streams are underway) so L1 starts immediately.
            w1_sb = cpool.tile([128, KC * F], BF16D, tag="w1")
            nc.sync.dma_start(w1_sb[:], w1[:])
            if has_bias:
                br_sb = cpool.tile([1, 2 * F + DOUT], FP32, tag="br")
                nc.sync.dma_start(br_sb[:], brows[:])
                ones_sb = cpool.tile([1, 128], FP32, tag="ones")
                nc.vector.memset(ones_sb[:], 1.0)

            w2_sb = cpool.tile([128, KC * F], BF16D, tag="w2")
            wa_sb = cpool.tile([128, KC * DOUT], BF16D, tag="wa")
            wb_sb = cpool.tile([128, KC * DOUT], BF16D, tag="wb")
            wc2_sb = cpool.tile([128, KC * DOUT], BF16D, tag="wc2")

            def late_loads():
                """Small load steps, drained a couple per L1 tile so they
                interleave with (rather than block) the msgs1 streams."""
                for t in range(4, NDT):
                    a, b = offgs[t] * DT, (offgs[t] + ETg[t]) * DT
                    yield lambda a=a, b=b: nc.sync.dma_start(
                        sg_sb[:, a:b], sg[:, a:b])
                yield lambda: nc.sync.dma_start(w2_sb[:], w2[:])
                if TGo:
                    yield lambda: nc.sync.dma_start(igo_sb[:], idxgo[:])
                yield lambda: nc.sync.dma_start(igr_sb[:], idxgr[:])
                yield lambda: nc.sync.dma_start(wa_sb[:], wa[:])
                yield lambda: nc.sync.dma_start(wb_sb[:], wb[:])
                yield lambda: nc.sync.dma_start(wc2_sb[:], wc2[:])
                if TCo:
                    yield lambda: nc.sync.dma_start(ico_sb[:], idxco[:])
                yield lambda: nc.sync.dma_start(icr_sb[:], idxcr[:])

            h2keep = kpool.tile([128, NDT, F], BF16D, tag="h2k")
            lapkeep = kpool.tile([128, NDT, DOUT], BF16D, tag="lap")

            first_gather = [0]
            qctr = [0]

            def scatter(src_local, src_full, eto, etr, io, ir, so,
                        idxo_sb, idxr_sb, s_sb, width=F,
                        msgs_tag="msgs", msgs_w=None, msgs_dt=BF16D):
                """One dest tile: gather own slabs from the Local staging
                tensor (no AllGather dependency — flows into the AG window)
                and remote slabs from the AllGathered table, into one msgs
                tile; one psum accumulation over [own | remote] S slabs."""
                mw = msgs_w if msgs_w is not None else ETMAX
                pool = mpool if msgs_tag == "msgs" else mpoolB
                msgs = pool.tile([128, mw, width], msgs_dt, tag=msgs_tag)

                def gathers(src, idx_sb, base, lo, n, nq, own):
                    bounds = [lo + n * i // nq for i in range(nq + 1)]
                    for a, b in zip(bounds[:-1], bounds[1:]):
                        if b <= a:
                            continue
                        # Queue 3 is reserved for own-slab gathers (no
                        # AllGather dependency) so they are never stuck
                        # behind a remote gather blocked at a queue head
                        # waiting for the collective.
                        if own:
                            q = 3
                        else:
                            q = qctr[0] % (3 if eto else 4)
                            qctr[0] += 1
                        gi = nc.gpsimd.dma_gather(
                            msgs[:, a:b, :], src[:],
                            idx_sb[:, (base + a - lo) * 8:
                                   (base + b - lo) * 8],
                            (b - a) * 128, (b - a) * 128, width,
                            single_packet=True, queue_num=q)
                        if first_gather[0] < 4:
                            add_dep_helper(gi.ins, lib.ins,
                                           reason="mlp lib before gather")
                            first_gather[0] += 1

                gathers(src_local, idxo_sb, io, 0, eto, 1, True)
                gathers(src_full, idxr_sb, ir, eto, etr,
                        min(3 if eto else 4, etr), False)
                # S slab as stationary lhsT, msgs streamed as rhs:
                # ps[d, f] += S[e, d].T @ msgs[e, f]  (node-major aggregate).
                ps = ppool3.tile([128, F], FP32, tag="psT")
                et = eto + etr
                for g in range(et):
                    nc.tensor.matmul(
                        ps[:, :width],
                        s_sb[:, (so + g) * DT:(so + g + 1) * DT],
                        msgs[:, g, :],
                        start=(g == 0),
                        stop=(g == et - 1))
                return ps

            def celu(z_ps, width, out_ap):
                """out = max(z,0) + min(exp(z)-1, 0); z read from PSUM."""
                e = wpool.tile([128, F], FP32, tag="e")
                nc.scalar.activation(e[:, :width], z_ps, Exp)
                nc.vector.tensor_scalar(
                    e[:, :width], e[:, :width], 1.0, 0.0,
                    Alu.subtract, Alu.min)
                nc.vector.scalar_tensor_tensor(
                    out_ap, z_ps, 0.0, e[:, :width], Alu.max, Alu.add)

            def gemm_bias(z_ps, width, b_off, stop=False):
                if has_bias:
                    nc.tensor.matmul(
                        z_ps, ones_sb[:],
                        br_sb[:, b_off:b_off + width],
                        start=False, stop=stop)

            def allgather(cin, cout):
                nc.gpsimd.collective_compute(
                    "AllGather", Alu.bypass,
                    replica_groups=[list(range(P))],
                    ins=[cin[:]],
                    outs=[cout[:]])

            import os
            PH = int(os.environ.get("GNN_PHASES", "9"))

            sgp_cm = tc.tile_pool(name="sgp", bufs=1)
            sgp = sgp_cm.__enter__()
            sg_sb = sgp.tile([128, TTg * DT], BF16D, tag="sg")
            # Only the first few S tiles load upfront; the rest drip through
            # late_loads so tile 0's msgs1 stream isn't queued behind 5.5MB
            # of S-table DMA on the sync engine.
            for t in range(4):
                a, b = offgs[t] * DT, (offgs[t] + ETg[t]) * DT
                nc.sync.dma_start(sg_sb[:, a:b], sg[:, a:b])

            def stream_scatter(t):
                """L1 variant of scatter: stream host-pregathered x rows
                (contiguous, static HWDGE DMA on two queues) + S matmuls."""
                from concourse.ap import AP as _AP
                o = offgs[t]
                et = ETg[t]
                msgs = mpool.tile([128, ETMAX, F], BF16D, tag="msgs")
                ha = (et + 1) // 2
                for eng, a, b in ((nc.sync, 0, ha), (nc.scalar, ha, et)):
                    if b <= a:
                        continue
                    eng.dma_start(
                        msgs[:, a:b, :],
                        _AP(msgs1, int(o + a) * 128 * F,
                            [[F, 128], [128 * F, b - a], [1, F]]))
                pst = ppool3.tile([128, F], FP32, tag="psT")
                for g in range(et):
                    nc.tensor.matmul(
                        pst[:],
                        sg_sb[:, (o + g) * DT:(o + g + 1) * DT],
                        msgs[:, g, :],
                        start=(g == 0), stop=(g == et - 1))
                return pst

            def gcn_tail(ps, w_sb, b_off):
                """agg -> transpose -> GEMM -> bias; returns psum z [128,F]."""
                agg = wpool.tile([128, F], BF16D, tag="agg")
                nc.vector.tensor_copy(agg[:], ps[:])
                tps = ppool.tile([128, KC, 128], BF16D, tag="tps")
                for k in range(KC):
                    nc.tensor.transpose(
                        tps[:, k, :], agg[:, k * 128:(k + 1) * 128], id_sb[:])
                aggT = wpool.tile([128, KC, 128], BF16D, tag="aggT")
                nc.vector.tensor_copy(aggT[:], tps[:])
                z = ppool.tile([128, F], FP32, tag="z")
                for k in range(KC):
                    nc.tensor.matmul(
                        z[:], aggT[:, k, :], w_sb[:, k * F:(k + 1) * F],
                        start=(k == 0), stop=False)
                gemm_bias(z[:], F, b_off, stop=True)
                return z

            # ---- layer 1: h1 = celu((Ag @ x) @ W1 + b1)
            _late = late_loads()
            for t in range(NDT):
                ps = stream_scatter(t)
                z = gcn_tail(ps, w1_sb, 0)
                h = wpool.tile([128, F], BF16D, tag="h")
                celu(z[:], F, h[:])
                nc.sync.dma_start(h1c[t * 128:(t + 1) * 128, :], h[:])
                if t >= 1:
                    for _ in range(3):
                        step = next(_late, None)
                        if step is not None:
                            step()
            for step in _late:
                step()
            if PH >= 2:
                allgather(h1c, h1f)

            # ---- layer 2: h2 = celu((Ag @ h1) @ W2 + b2), kept on chip.
            #      Per tile, also transpose h2 to feature-major and project
            #      pq = h2 @ [Wb | Wc2] for the cheb passes.
            if PH >= 3:
                for t in range(NDT):
                    ps = scatter(h1c, h1f, ETgo[t], ETgr[t], int(offgo[t]),
                                 int(offgr[t]), int(offgs[t]),
                                 igo_sb, igr_sb, sg_sb)
                    z = gcn_tail(ps, w2_sb, F)
                    h2t = wpool.tile([128, F], BF16D, tag="h")
                    celu(z[:], F, h2t[:])
                    # feature-major h2 for the three h2 @ W GEMM terms
                    tps2 = ppool.tile([128, KC, 128], BF16D, tag="tps")
                    for k in range(KC):
                        nc.tensor.transpose(
                            tps2[:, k, :], h2t[:, k * 128:(k + 1) * 128],
                            id_sb[:])
                    nc.vector.tensor_copy(h2keep[:, t, :], tps2[:])
                    if PH >= 4:
                        # pq = h2 @ [Wb | Wc2]  (node-major in psum)
                        zpq = ppool.tile([128, F], FP32, tag="z")
                        for k in range(KC):
                            nc.tensor.matmul(
                                zpq[:, 0:DOUT],
                                h2keep[:, t, k * 128:(k + 1) * 128],
                                wb_sb[:, k * DOUT:(k + 1) * DOUT],
                                start=(k == 0), stop=(k == KC - 1))
                        for k in range(KC):
                            nc.tensor.matmul(
                                zpq[:, DOUT:2 * DOUT],
                                h2keep[:, t, k * 128:(k + 1) * 128],
                                wc2_sb[:, k * DOUT:(k + 1) * DOUT],
                                start=(k == 0), stop=(k == KC - 1))
                        pq = wpool.tile([128, F], FP8D, tag="h8")
                        nc.vector.tensor_copy(pq[:], zpq[:])
                        nc.sync.dma_start(pqc[t * 128:(t + 1) * 128, :], pq[:])
                if PH >= 4:
                    allgather(pqc, pqf)

            # sg is dead after L2 -- release its SBUF zone and put the cheb S
            # tiles there (loads overlap the pq AllGather window).
            sgp_cm.__exit__(None, None, None)
            scp_cm = tc.tile_pool(name="scp", bufs=1)
            scp = scp_cm.__enter__()
            sc_sb = scp.tile([128, TTc * DT], BF16D, tag="sc")
            for t in range(NDT):
                a = offcs[t] * DT
                b = (offcs[t] + ETco[t] + ETcr[t]) * DT
                nc.sync.dma_start(sc_sb[:, a:b], sc[:, a:b])
            # ---- cheb pass A: [lhat_p | lhat_q] = lhat(pq); keep lhat_p,
            #      AllGather r = lhat_q (256 wide).
            if PH >= 5:
                for t in range(NDT):
                    ps = scatter(pqc, pqf, ETco[t], ETcr[t], int(offco[t]),
                                 int(offcr[t]), int(offcs[t]),
                                 ico_sb, icr_sb, sc_sb,
                                 msgs_tag="msgs256", msgs_w=ETMAX,
                                 msgs_dt=FP8D)
                    nc.vector.tensor_copy(lapkeep[:, t, :], ps[:, 0:DOUT])
                    rt = wpool.tile([128, DOUT], FP8D, tag="rt")
                    nc.vector.tensor_copy(rt[:], ps[:, DOUT:2 * DOUT])
                    nc.sync.dma_start(rc[t * 128:(t + 1) * 128, :], rt[:])
                allgather(rc, rf)

            # ---- cheb pass B + output:
            # out = celu(lhat(r) + h2 @ Wa + lhat_p + bc)
            if PH >= 6:
                # Local terms zoL = h2 @ Wa + lhat_p + bc precomputed on PE
                # while the r AllGather is in flight.
                zokeep = kpool.tile([128, NDT, DOUT], FP32, tag="zok")
                for t in range(NDT):
                    zl = ppool.tile([128, F], FP32, tag="z")
                    zv = zl[:, :DOUT]
                    for k in range(KC):
                        nc.tensor.matmul(
                            zv, h2keep[:, t, k * 128:(k + 1) * 128],
                            wa_sb[:, k * DOUT:(k + 1) * DOUT],
                            start=(k == 0), stop=False)
                    nc.tensor.matmul(
                        zv, id8_sb[:], lapkeep[:, t, :],
                        start=False, stop=not has_bias)
                    gemm_bias(zv, DOUT, 2 * F, stop=True)
                    nc.vector.tensor_copy(zokeep[:, t, :], zv)
                for t in range(NDT):
                    ps = scatter(rc, rf, ETco[t], ETcr[t], int(offco[t]),
                                 int(offcr[t]), int(offcs[t]),
                                 ico_sb, icr_sb, sc_sb,
                                 width=DOUT, msgs_tag="msgs256",
                                 msgs_w=ETCMAX, msgs_dt=FP8D)
                    zs = wpool.tile([128, DOUT], FP32, tag="zs")
                    nc.vector.scalar_tensor_tensor(
                        zs[:], ps[:, :DOUT], 1.0, zokeep[:, t, :],
                        Alu.mult, Alu.add)
                    of = wpool.tile([128, DOUT], FP32, tag="of")
                    celu(zs[:], DOUT, of[:])
                    nc.sync.dma_start(outp[t * 128:(t + 1) * 128, :], of[:])

            scp_cm.__exit__(None, None, None)

    nc.compile()
    _CACHE[key] = nc
    return nc


# ------------------------------------------------------------------- driver

def _run(inputs, trace=False, tmpdir=None):
    ETs, biases, in_maps = _prep(**inputs)
    has_bias = any(np.any(b != 0) for b in biases)
    if has_bias:
        brow = np.concatenate(biases).astype(np.float32)[None, :]
        for m in in_maps:
            m["brows"] = brow
    nc = _build_program(ETs, has_bias)
    res = run_bass_kernel_spmd(nc, in_maps, core_ids=list(range(P)),
                               trace=trace, tmpdir=tmpdir)
    out = np.concatenate(
        [res.results[c]["out"][:NPC] for c in range(P)], axis=0)
    return out.astype(np.float32), res


def kernel(**inputs) -> np.ndarray:
    out, _ = _run(inputs)
    return out
